# revision 24
# baseline (speedup 1.0000x reference)
"""Trainium2 Bass kernel for a dense transformer block (B=4,T=2048,H=16,D=64,C=1024,FF=4096).

Sharding: batch b -> core pair (2b, 2b+1). Within a pair, attention is split
by heads (8 heads/core, Megatron column-parallel QKV + row-parallel W_o); the
W_o partials are combined with a per-q-slice pair ReduceScatter and each core
runs LN2 + the full-FF MLP on its half (1024) of the rows. Output rows are
disjoint across cores; the host concatenates.

Key scheduling ideas vs the naive phase-by-phase version:
- LN1 / z / z^T production is interleaved with the QKV matmuls per 512-token
  slice so the tensor engine never idles long enough for the HAM clock gate
  to re-throttle it. The first x tile is DMA'd before the weight tensors.
- Attention processes heads in pairs: the two heads' score blocks land in one
  [P, 1024] PSUM tile (2 banks) so a single Exp activation covers both,
  halving the per-instruction ACT overhead that otherwise gates the AV
  matmuls. Within a pair the AV matmuls trail the score matmuls by AVLAG kc
  blocks, so the in-order tensor queue always has runnable work while the
  activation engine streams exps (the attention wall-clock floor).
- The k>q (masked) column prefix of diagonal-band blocks is never computed,
  exp'd, or consumed.
- K bias is never applied: softmax(q'.(k+bk)) == softmax(q'.k) exactly, since
  q'.bk is constant along the key axis. Q bias is applied on the PSUM->SBUF
  eviction. LN affines are folded into the matmul weights on the host.
- Softmax normalization: the ones-column of V yields the denominator row; a
  K=1 ones-matmul broadcasts it across partitions in PSUM and one reciprocal
  + multiply normalizes both heads. Nothing here touches GpSimd, whose queue
  carries the (blocking) collective waits.
- W_o partials for slice s are computed eagerly at the end of slice s (the
  tensor engine has slack in the ACT-bound attention) so each ReduceScatter
  fires as early as possible; b_o rides in the W_o accumulation as a K=1
  ones-row matmul (host zeroes it on one core of the pair). 2-rank RS is
  30-100us/chunk and the trigger+wait chain serializes on gpsimd, so no
  engine-queue work may depend on an RS inside the attention scope.
- Phase B is ordered so tensor work covers the RS waits: residual+LN2 for
  chunks 0-1 (RS long done), z2, FC first half, W_out first half (needs only
  FC half 0), then chunks 2-3, FC+W_out second halves. b_out rides in the
  W_out accumulators as a K=1 ones-row matmul.
"""
import math

import ml_dtypes
import numpy as np

P = 128
B, T, H, D = 4, 2048, 16, 64
C = H * D
FF = 4096
EPS = 1e-5
N_CORES = 8

_CACHE = {}
LAST_RESULT = None


def _build(T, C, H, D, FF, n_cores, groups, phase_limit=99, sim_safe=False,
           debug_outs=False):
    """Build + compile the single-core SPMD program. Returns the Bacc object."""
    from contextlib import ExitStack

    import concourse.mybir as mybir
    import concourse.tile as tile
    from concourse import bacc

    dt = mybir.dt
    AF = mybir.ActivationFunctionType
    OP = mybir.AluOpType

    HH = H // 2               # heads per core
    QH = HH * D               # per-core c_out for each of q,k,v
    NQH = QH // P
    NT = T // P
    T2 = T // 2               # own rows
    NT2 = T2 // P
    NC = C // P
    NF = FF // P
    SL = min(512, T)          # attention q-slice width
    NSL = T // SL
    DBLK = SL // P            # 128-blocks per q-slice
    HPC = P // D              # heads per 128-partition chunk (=2)
    TSW = min(512, T)         # qkv t-slice width
    NTS = T // TSW
    CSW = min(512, C)
    NCS = C // CSW
    TS2 = min(512, T2)
    NB = 4 if NC % 4 == 0 else 1  # transposes batched per psum bank
    CPTS = TSW // P           # token chunks per t-slice (=4)
    NPAIR = HH // 2           # head pairs (=4)
    WQ = FF // 4              # wfc quarter width (=1024)
    HS = SL // 2              # tokens owned per core per q-slice (=256)
    CPC = NT2 // NSL          # own-row P-chunks per q-slice (=2)
    assert QH % P == 0 and T % SL == 0 and SL % P == 0 and HPC == 2

    nc = bacc.Bacc("TRN2", target_bir_lowering=False, debug=False,
                   num_devices=n_cores)
    gelu_af = (mybir.ActivationFunctionType.Identity if sim_safe
               else mybir.ActivationFunctionType.Gelu)

    # ---- kernel I/O ----
    x_full = nc.dram_tensor("x_full", [T, C], dt.bfloat16, kind="ExternalInput")
    x_own = nc.dram_tensor("x_own", [T2, C], dt.float32, kind="ExternalInput")
    wq = nc.dram_tensor("wq", [C, QH], dt.bfloat16, kind="ExternalInput")
    wk = nc.dram_tensor("wk", [C, QH], dt.bfloat16, kind="ExternalInput")
    wv = nc.dram_tensor("wv", [C, QH], dt.bfloat16, kind="ExternalInput")
    bq = nc.dram_tensor("bq", [QH], dt.float32, kind="ExternalInput")
    bv = nc.dram_tensor("bv", [QH], dt.float32, kind="ExternalInput")
    wo = nc.dram_tensor("wo", [QH, C], dt.bfloat16, kind="ExternalInput")
    bo = nc.dram_tensor("bo", [C], dt.bfloat16, kind="ExternalInput")
    wfc = nc.dram_tensor("wfc", [C, FF], dt.bfloat16, kind="ExternalInput")
    bfc = nc.dram_tensor("bfc", [FF], dt.float32, kind="ExternalInput")
    wout = nc.dram_tensor("wout", [FF, C], dt.bfloat16, kind="ExternalInput")
    bout = nc.dram_tensor("bout", [C], dt.bfloat16, kind="ExternalInput")
    tri = nc.dram_tensor("tri", [P, P], dt.bfloat16, kind="ExternalInput")
    ident = nc.dram_tensor("ident", [P, P], dt.bfloat16, kind="ExternalInput")
    out = nc.dram_tensor("out", [T2, C], dt.float32, kind="ExternalOutput")

    # collective bounce buffers (internal DRAM)
    r_bounce = nc.dram_tensor("r_bounce", [T, C], dt.bfloat16)
    r_own_b = nc.dram_tensor("r_own_b", [T2, C], dt.bfloat16)

    dbg = {}
    if debug_outs:
        HH_ = H // 2
        for nm, shp, dt_ in (
                ("dbg_zt", [P, C // P, T], dt.bfloat16),
                ("dbg_kt", [P, (HH_ * D) // P, T], dt.bfloat16),
                ("dbg_qt", [P, (HH_ * D) // P, T], dt.bfloat16),
                ("dbg_v", [P, T // P, HH_, D + 1], dt.bfloat16),
                ("dbg_recb", [D, 2 * min(512, T)], dt.float32),
                ("dbg_yt", [P, (HH_ * D) // P, min(512, T)], dt.bfloat16),
                ("dbg_st1", [P, T // P, 2], dt.float32),
                ("dbg_x2", [P, T // (2 * P), C], dt.bfloat16),
                ("dbg_z2t", [P, C // P, T // 2], dt.bfloat16),
        ):
            dbg[nm] = nc.dram_tensor(nm, shp, dt_, kind="ExternalOutput")

    x_r = x_full.rearrange("(i p) c -> p i c", p=P)
    xo_r = x_own.rearrange("(i p) c -> p i c", p=P)
    out_r = out.rearrange("(i p) c -> p i c", p=P)
    rb_r = r_bounce.rearrange("(i p) c -> p i c", p=P)
    rob_r = r_own_b.rearrange("(i p) c -> p i c", p=P)

    inv_sqrt_d = 1.0 / math.sqrt(D)

    with tile.TileContext(nc) as tc, ExitStack() as stk:
        pool_const = stk.enter_context(tc.tile_pool(name="const", bufs=1))

        tri2_sb = pool_const.tile([P, 2, P], dt.bfloat16)
        id_sb = pool_const.tile([P, P], dt.bfloat16)
        nc.sync.dma_start(tri2_sb[:, 0, :], tri[:])
        nc.sync.dma_start(tri2_sb[:, 1, :], tri[:])
        nc.sync.dma_start(id_sb[:], ident[:])
        bq_sb = pool_const.tile([P, NQH], dt.float32)
        bv_row = pool_const.tile([1, QH], dt.float32)
        bo_bf = pool_const.tile([1, C], dt.bfloat16)
        bfc_sb = pool_const.tile([P, NF], dt.float32)
        bout_bf = pool_const.tile([1, C], dt.bfloat16)
        eps_sb = pool_const.tile([P, 1], dt.float32)
        nc.vector.memset(eps_sb[:], EPS)
        ones1 = pool_const.tile([1, P], dt.float32)
        nc.vector.memset(ones1[:], 1.0)
        ones_bf = pool_const.tile([1, P], dt.bfloat16)
        nc.vector.memset(ones_bf[:], 1.0)
        bv_full = pool_const.tile([P, QH], dt.float32)
        nc.sync.dma_start(bq_sb[:], bq.rearrange("(a p) -> p a", p=P))
        nc.sync.dma_start(bv_row[:], bv[None, :])
        nc.sync.dma_start(bo_bf[:], bo[None, :])
        nc.sync.dma_start(bfc_sb[:], bfc.rearrange("(a p) -> p a", p=P))
        nc.sync.dma_start(bout_bf[:], bout[None, :])

        def ln_stats_chunk(pool_tmp, st, i, xc):
            """bn_stats/aggr for one [P, C] chunk -> st[:, i, 0]=mean, 1=var."""
            bns = pool_tmp.tile([P, 2, 6], dt.float32, tag="bns",
                                name=f"bns_{i}")
            xv = xc.rearrange("p (a b) -> p a b", a=2)
            nc.vector.bn_stats(bns[:, 0, :], xv[:, 0, :])
            nc.vector.bn_stats(bns[:, 1, :], xv[:, 1, :])
            nc.vector.bn_aggr(st[:, i, :], bns[:])

        def ln_finalize(pool_tmp, st, rstd, nmr, i0, n, nm):
            """Batched rstd / -mean*rstd for chunks [i0, i0+n)."""
            vtmp = pool_tmp.tile([P, NT], dt.float32, tag="vtmp",
                                 name=f"vt_{nm}")
            nc.scalar.activation(vtmp[:, i0 : i0 + n], st[:, i0 : i0 + n, 1],
                                 AF.Sqrt, bias=eps_sb[:])
            nc.vector.reciprocal_approx_fast(rstd[:, i0 : i0 + n],
                                             vtmp[:, i0 : i0 + n])
            nc.vector.scalar_tensor_tensor(
                nmr[:, i0 : i0 + n], st[:, i0 : i0 + n, 0], -1.0,
                rstd[:, i0 : i0 + n], OP.mult, OP.mult)

        # pools that outlive the attention scope: residual/LN2 state for the
        # second half of the block
        pool_ph4 = stk.enter_context(tc.tile_pool(name="pph4", bufs=1))
        X2 = pool_ph4.tile([P, NT2, C], dt.bfloat16, tag="x2")
        Z2T = pool_ph4.tile([P, NC, T2], dt.bfloat16, tag="z2t")
        st2 = pool_ph4.tile([P, NT2, 2], dt.float32, tag="st2")
        rstd2 = pool_ph4.tile([P, NT2], dt.float32, tag="rstd2")
        nmr2 = pool_ph4.tile([P, NT2], dt.float32, tag="nmr2")
        pool_xo = stk.enter_context(tc.tile_pool(name="pxo", bufs=5))
        xo_tiles = {}

        def xo_fetch(i):
            xoc = pool_xo.tile([P, C], dt.float32, tag="xoc",
                               name=f"xo_{i}")
            nc.sync.dma_start(xoc[:], xo_r[:, i, :])
            xo_tiles[i] = xoc

        # attn persistents open first so everything transient frees above them
        with tc.tile_pool(name="pattn", bufs=1) as pool_attn:
            QT = pool_attn.tile([P, NQH, T], dt.bfloat16, tag="QT")
            KT = pool_attn.tile([P, NQH, T], dt.bfloat16, tag="KT")
            V = pool_attn.tile([P, NT, HH, D + 1], dt.bfloat16, tag="V")
            wo_sb = pool_attn.tile([P, NQH, C], dt.bfloat16, tag="wo")

            # ===== phase A: x -> LN1 -> z -> z^T -> QKV, per t-slice =====
            with ExitStack() as es_zt:
                pool_zt = es_zt.enter_context(tc.tile_pool(name="pzt", bufs=1))
                ZT = pool_zt.tile([P, NC, T], dt.bfloat16)
                pool_wqkv = es_zt.enter_context(tc.tile_pool(name="pw1",
                                                             bufs=1))
                wq_sb = pool_wqkv.tile([P, NC, QH], dt.bfloat16, tag="wq")
                wk_sb = pool_wqkv.tile([P, NC, QH], dt.bfloat16, tag="wk")
                wv_sb = pool_wqkv.tile([P, NC, QH], dt.bfloat16, tag="wv")

                with tc.tile_pool(name="pstat", bufs=1) as pool_stat, \
                     tc.tile_pool(name="pxs", bufs=2) as pool_xs, \
                     tc.tile_pool(name="ps_tra", bufs=2, space="PSUM") as ps_tra, \
                     tc.tile_pool(name="ps_mm1", bufs=4, space="PSUM") as ps_mm1:
                    # first x tile BEFORE the 3MB of weight DMAs so LN work
                    # starts immediately
                    xgs = [None] * NTS
                    xgs[0] = pool_xs.tile([P, CPTS, C], dt.bfloat16, tag="xg",
                                          name="xg_0")
                    for j in range(CPTS):
                        nc.sync.dma_start(xgs[0][:, j, :], x_r[:, j, :])
                    nc.sync.dma_start(wk_sb[:],
                                      wk.rearrange("(ci p) o -> p ci o", p=P))
                    nc.sync.dma_start(wq_sb[:],
                                      wq.rearrange("(ci p) o -> p ci o", p=P))
                    nc.sync.dma_start(wv_sb[:],
                                      wv.rearrange("(ci p) o -> p ci o", p=P))
                    nc.sync.dma_start(wo_sb[:],
                                      wo.rearrange("(ci p) o -> p ci o", p=P))
                    # broadcast bv row -> full tile via ones-column matmul
                    pbv = ps_mm1.tile([P, 512], dt.float32, tag="mmp",
                                      name="bc_bv")
                    nc.tensor.matmul(pbv[:, :QH], ones1[:], bv_row[:])
                    nc.vector.tensor_copy(bv_full[:], pbv[:, :QH])

                    st1 = pool_stat.tile([P, NT, 2], dt.float32, tag="st1")
                    rstd1 = pool_stat.tile([P, NT], dt.float32, tag="rstd1")
                    nmr1 = pool_stat.tile([P, NT], dt.float32, tag="nmr1")

                    for ts_ in range(NTS):
                        i0 = ts_ * CPTS
                        if ts_ + 1 < NTS:
                            xgs[ts_ + 1] = pool_xs.tile(
                                [P, CPTS, C], dt.bfloat16, tag="xg",
                                name=f"xg_{ts_ + 1}")
                            for j in range(CPTS):
                                nc.sync.dma_start(
                                    xgs[ts_ + 1][:, j, :],
                                    x_r[:, i0 + CPTS + j, :])
                        xg = xgs[ts_]
                        for j in range(CPTS):
                            ln_stats_chunk(pool_stat, st1, i0 + j,
                                           xg[:, j, :])
                        ln_finalize(pool_stat, st1, rstd1, nmr1, i0, CPTS,
                                    f"a{ts_}")
                        for j in range(CPTS):
                            i = i0 + j
                            zc = pool_xs.tile([P, C], dt.bfloat16, tag="zc")
                            nc.scalar.activation(zc[:], xg[:, j, :],
                                                 AF.Identity,
                                                 bias=nmr1[:, i : i + 1],
                                                 scale=rstd1[:, i : i + 1])
                            for jj in range(NC // NB):
                                pt = ps_tra.tile([P, NB * P], dt.bfloat16,
                                                 tag="trp")
                                for j4 in range(NB):
                                    cjj = jj * NB + j4
                                    nc.tensor.transpose(
                                        pt[:, j4 * P : (j4 + 1) * P],
                                        zc[:, cjj * P : (cjj + 1) * P],
                                        id_sb[:])
                                nc.vector.tensor_copy(
                                    ZT[:, jj * NB : (jj + 1) * NB,
                                       i * P : (i + 1) * P],
                                    pt[:].rearrange("p (a b) -> p a b", a=NB))

                        # QKV for this t-slice
                        sl_t = slice(ts_ * TSW, (ts_ + 1) * TSW)
                        for co in range(NQH):
                            pm = ps_mm1.tile([P, TSW], dt.float32, tag="mmp",
                                             name=f"k_{ts_}_{co}")
                            for ci in range(NC):
                                nc.tensor.matmul(
                                    pm[:], wk_sb[:, ci, co * P : (co + 1) * P],
                                    ZT[:, ci, sl_t],
                                    start=(ci == 0), stop=(ci == NC - 1))
                            # no K bias: softmax-invariant (cancels exactly)
                            nc.vector.tensor_copy(KT[:, co, sl_t], pm[:])
                        for co in range(NQH):
                            pm = ps_mm1.tile([P, TSW], dt.float32, tag="mmp",
                                             name=f"q_{ts_}_{co}")
                            for ci in range(NC):
                                nc.tensor.matmul(
                                    pm[:], wq_sb[:, ci, co * P : (co + 1) * P],
                                    ZT[:, ci, sl_t],
                                    start=(ci == 0), stop=(ci == NC - 1))
                            nc.scalar.activation(QT[:, co, sl_t], pm[:],
                                                 AF.Identity,
                                                 bias=bq_sb[:, co : co + 1])
                        for j in range(CPTS):
                            ti = i0 + j
                            pm = ps_mm1.tile([P, QH], dt.float32, tag="mmp",
                                             name=f"v_{ts_}_{j}")
                            for ci in range(NC):
                                nc.tensor.matmul(
                                    pm[:], ZT[:, ci, ti * P : (ti + 1) * P],
                                    wv_sb[:, ci, :],
                                    start=(ci == 0), stop=(ci == NC - 1))
                            nc.vector.tensor_tensor(
                                V[:, ti, :, :D],
                                pm[:].rearrange("p (h d) -> p h d", d=D),
                                bv_full[:].rearrange("p (h d) -> p h d", d=D),
                                OP.add)
                    nc.vector.memset(V[:, :, :, D], 1.0)
                    if debug_outs:
                        nc.sync.dma_start(dbg["dbg_zt"][:], ZT[:])
                        nc.sync.dma_start(dbg["dbg_st1"][:], st1[:])
                if debug_outs:
                    nc.sync.dma_start(dbg["dbg_kt"][:], KT[:])
                    nc.sync.dma_start(dbg["dbg_qt"][:], QT[:])
                    nc.sync.dma_start(dbg["dbg_v"][:], V[:])

            # ===== attention: head pairs, batched exp, kc-pipelined AV =====
            AVLAG = 3          # AV trails scores by this many kc blocks
            with tc.tile_pool(name="ppt", bufs=5) as pool_pt, \
                 tc.tile_pool(name="pyt", bufs=2) as pool_yt, \
                 tc.tile_pool(name="prs", bufs=3) as pool_rs, \
                 tc.tile_pool(name="prec", bufs=1) as pool_rec, \
                 tc.tile_pool(name="prec2", bufs=2) as pool_rec2, \
                 tc.tile_pool(name="ps_s", bufs=2, space="PSUM") as ps_s, \
                 tc.tile_pool(name="ps_o", bufs=2, space="PSUM") as ps_o:
                for s in range(NSL if phase_limit >= 2 else 0):
                    kc_max = (s + 1) * DBLK
                    yts = pool_yt.tile([P, NQH, SL], dt.bfloat16,
                                       tag="yts", name=f"yts_{s}")
                    for hp in range(NPAIR):
                        # heads (2hp, 2hp+1) live in rows (0:64, 64:128) of
                        # partition-chunk hp of QT/KT and yts.
                        po = ps_o.tile([P, 2 * SL], dt.float32, tag="op",
                                       name=f"po_{s}_{hp}")
                        ptbs = [None] * kc_max

                        def av_mm(kc):
                            c0a = max(kc - s * DBLK, 0) * P
                            for e in range(2):
                                nc.tensor.matmul(
                                    po[: D + 1, e * SL + c0a : (e + 1) * SL],
                                    V[:, kc, 2 * hp + e, :],
                                    ptbs[kc][:, e, c0a:],
                                    start=(kc == 0),
                                    stop=(kc == kc_max - 1))

                        for kc in range(kc_max):
                            c0 = max(kc - s * DBLK, 0) * P
                            pm = ps_s.tile([P, 2 * SL], dt.float32, tag="sp",
                                           name=f"sc_{s}_{hp}_{kc}")
                            for e in range(2):
                                hr = slice(e * D, (e + 1) * D)
                                nc.tensor.matmul(
                                    pm[:, e * SL + c0 : (e + 1) * SL],
                                    KT[hr, hp, kc * P : (kc + 1) * P],
                                    QT[hr, hp, s * SL + c0 : (s + 1) * SL],
                                    start=True, stop=True)
                            ptb = pool_pt.tile([P, 2, SL], dt.bfloat16,
                                               tag="ptb",
                                               name=f"ptb_{s}_{hp}_{kc}")
                            ptbs[kc] = ptb
                            pmv = pm[:].rearrange("p (e q) -> p e q", e=2)
                            nc.scalar.activation(
                                ptb[:, :, c0:], pmv[:, :, c0:],
                                AF.Exp, scale=inv_sqrt_d)
                            if kc >= s * DBLK:
                                # mask the diagonal 128x128 sub-block
                                nc.vector.tensor_tensor(
                                    ptb[:, :, c0 : c0 + P],
                                    ptb[:, :, c0 : c0 + P],
                                    tri2_sb[:], OP.mult)
                            if kc >= AVLAG:
                                av_mm(kc - AVLAG)
                        for kc in range(max(kc_max - AVLAG, 0), kc_max):
                            av_mm(kc)
                        # normalization: den row -> ones-matmul broadcast
                        # into PSUM -> reciprocal -> mult (no gpsimd: the
                        # collective waits live there)
                        denb = pool_rec.tile([1, 2 * SL], dt.bfloat16,
                                             tag="denb", name=f"dnb_{s}_{hp}")
                        nc.vector.tensor_copy(denb[:], po[D : D + 1, :])
                        pb = ps_o.tile([P, 2 * SL], dt.float32, tag="op",
                                       name=f"pb_{s}_{hp}")
                        for e in range(2):
                            nc.tensor.matmul(
                                pb[:D, e * SL : (e + 1) * SL],
                                ones_bf[:, :D],
                                denb[:, e * SL : (e + 1) * SL],
                                start=True, stop=True)
                        recb = pool_rec2.tile([D, 2 * SL], dt.float32,
                                              tag="recb", name=f"rb_{s}_{hp}")
                        nc.vector.reciprocal_approx_fast(recb[:], pb[:D, :])
                        for e in range(2):
                            nc.vector.tensor_tensor(
                                yts[e * D : (e + 1) * D, hp, :],
                                po[:D, e * SL : (e + 1) * SL],
                                recb[:, e * SL : (e + 1) * SL], OP.mult)
                        if debug_outs and s == 0 and hp == 0:
                            nc.sync.dma_start(dbg["dbg_recb"][:], recb[:])
                    if debug_outs and s == 0:
                        nc.sync.dma_start(dbg["dbg_yt"][:], yts[:])
                    # W_o partials for this slice NOW (tensor has slack in the
                    # ACT-bound attention), so the RS fires as early as
                    # possible; its ~30us runs under the following slices.
                    for tis in range(SL // P):
                        ti = s * (SL // P) + tis
                        pos = ti
                        r_sb = pool_rs.tile([P, C], dt.bfloat16, tag="rsb",
                                            name=f"rsb_{s}_{tis}")
                        for cs in range(NCS):
                            pm = ps_s.tile([P, 2 * CSW], dt.float32,
                                           tag="sp", name=f"wo_{s}_{tis}_{cs}")
                            # b_o init (host zeroes it on one core of the
                            # pair; the RS sums both)
                            nc.tensor.matmul(
                                pm[:, :CSW], ones_bf[:],
                                bo_bf[:, cs * CSW : (cs + 1) * CSW],
                                start=True, stop=False)
                            for ci in range(NQH):
                                nc.tensor.matmul(
                                    pm[:, :CSW],
                                    yts[:, ci, tis * P : (tis + 1) * P],
                                    wo_sb[:, ci, cs * CSW : (cs + 1) * CSW],
                                    start=False, stop=(ci == NQH - 1))
                            nc.vector.tensor_copy(
                                r_sb[:, cs * CSW : (cs + 1) * CSW],
                                pm[:, :CSW])
                        nc.sync.dma_start(rb_r[:, pos, :], r_sb[:])
                    if s < 2 and phase_limit >= 5:
                        # prefetch this slice's residual rows (no deps)
                        for cc in range(CPC):
                            xo_fetch(s * CPC + cc)
                    if phase_limit >= 4:
                        nc.gpsimd.collective_compute(
                            "ReduceScatter", OP.add, replica_groups=groups,
                            ins=[r_bounce[s * SL : (s + 1) * SL, :].opt()],
                            outs=[r_own_b[s * (SL // 2) :
                                          (s + 1) * (SL // 2), :].opt()])

        # ===== phase B: W_o (full contraction, own rows) + residual + LN2
        # ===== + FC/gelu + W_out
        with tc.tile_pool(name="pht", bufs=1) as pool_ht:
            HT = pool_ht.tile([P, NF, T2], dt.bfloat16)

            with ExitStack() as es_z2t:
                pool_wfc = es_z2t.enter_context(
                    tc.tile_pool(name="pwfc", bufs=2))
                pool_ro = es_z2t.enter_context(
                    tc.tile_pool(name="pro", bufs=2))
                ps_h = es_z2t.enter_context(
                    tc.tile_pool(name="ps_h", bufs=4, space="PSUM"))
                pool_wout = es_z2t.enter_context(
                    tc.tile_pool(name="pwout", bufs=3))
                pool_out = es_z2t.enter_context(
                    tc.tile_pool(name="pout", bufs=3))

                with tc.tile_pool(name="pz2c", bufs=2) as pool_z2c, \
                     tc.tile_pool(name="ps_out", bufs=4,
                                  space="PSUM") as ps_out:
                    def phase4_pre(cc):
                        """Residual add + LN2 stats for RS chunk cc."""
                        for j in range(CPC):
                            i = cc * CPC + j
                            if i not in xo_tiles:
                                xo_fetch(i)
                            xoc = xo_tiles[i]
                            roc = pool_ro.tile([P, C], dt.bfloat16,
                                               tag="roc", name=f"ro_{i}")
                            nc.sync.dma_start(roc[:], rob_r[:, i, :])
                            nc.vector.tensor_tensor(X2[:, i, :], xoc[:],
                                                    roc[:], OP.add)
                            ln_stats_chunk(pool_ph4, st2, i, X2[:, i, :])

                    def z2_chunk(i):
                        z2c = pool_z2c.tile([P, C], dt.bfloat16, tag="z2c",
                                            name=f"z2_{i}")
                        nc.scalar.activation(z2c[:], X2[:, i, :], AF.Identity,
                                             bias=nmr2[:, i : i + 1],
                                             scale=rstd2[:, i : i + 1])
                        for jj in range(NC // NB):
                            pt = ps_h.tile([P, NB * P], dt.bfloat16,
                                           tag="hp")
                            for j4 in range(NB):
                                cjj = jj * NB + j4
                                nc.tensor.transpose(
                                    pt[:, j4 * P : (j4 + 1) * P],
                                    z2c[:, cjj * P : (cjj + 1) * P], id_sb[:])
                            nc.vector.tensor_copy(
                                Z2T[:, jj * NB : (jj + 1) * NB,
                                    i * P : (i + 1) * P],
                                pt[:].rearrange("p (a b) -> p a b", a=NB))

                    def fc_ts(ts2):
                        sl2 = slice(ts2 * TS2, (ts2 + 1) * TS2)
                        for q in range(FF // WQ):
                            wfc_sb = pool_wfc.tile(
                                [P, NC, WQ], dt.bfloat16, tag="wfc",
                                name=f"wfc_{ts2}_{q}")
                            nc.sync.dma_start(
                                wfc_sb[:],
                                wfc[:, q * WQ : (q + 1) * WQ]
                                .rearrange("(ci p) o -> p ci o", p=P))
                            for f in range(WQ // P):
                                fg = q * (WQ // P) + f
                                pm = ps_h.tile([P, TS2], dt.float32,
                                               tag="hp",
                                               name=f"h_{ts2}_{fg}")
                                for ci in range(NC):
                                    nc.tensor.matmul(
                                        pm[:],
                                        wfc_sb[:, ci, f * P : (f + 1) * P],
                                        Z2T[:, ci, sl2],
                                        start=(ci == 0),
                                        stop=(ci == NC - 1))
                                nc.scalar.activation(
                                    HT[:, fg, sl2], pm[:], gelu_af,
                                    bias=bfc_sb[:, fg : fg + 1])

                    def wout_half(th):
                        """W_out + residual for ti in [4*th, 4*th+4)."""
                        tis = range(4 * th, 4 * th + 4)
                        for cs in range(NCS):
                            pms = {}
                            for ti in tis:
                                pms[ti] = ps_out.tile(
                                    [P, CSW], dt.float32, tag="outp",
                                    name=f"outp_{th}_{cs}_{ti}")
                                nc.tensor.matmul(
                                    pms[ti][:], ones_bf[:],
                                    bout_bf[:, cs * CSW : (cs + 1) * CSW],
                                    start=True, stop=False)
                            for fi in range(NF):
                                wout_sb = pool_wout.tile(
                                    [P, CSW], dt.bfloat16, tag="wout",
                                    name=f"wout_{th}_{cs}_{fi}")
                                nc.sync.dma_start(
                                    wout_sb[:],
                                    wout[fi * P : (fi + 1) * P,
                                         cs * CSW : (cs + 1) * CSW])
                                for ti in tis:
                                    nc.tensor.matmul(
                                        pms[ti][:],
                                        HT[:, fi, ti * P : (ti + 1) * P],
                                        wout_sb[:],
                                        start=False, stop=(fi == NF - 1))
                            for ti in tis:
                                o_sb = pool_out.tile([P, CSW], dt.float32,
                                                     tag="osb",
                                                     name=f"o_{th}_{cs}_{ti}")
                                nc.vector.tensor_tensor(
                                    o_sb[:], pms[ti][:],
                                    X2[:, ti, cs * CSW : (cs + 1) * CSW],
                                    OP.add)
                                nc.sync.dma_start(
                                    out_r[:, ti, cs * CSW : (cs + 1) * CSW],
                                    o_sb[:])

                    if phase_limit >= 5:
                        phase4_pre(0)
                        phase4_pre(1)
                        ln_finalize(pool_ph4, st2, rstd2, nmr2, 0, 2 * CPC,
                                    "c01")
                        for i in range(2 * CPC):
                            z2_chunk(i)
                    if phase_limit >= 6:
                        fc_ts(0)
                        if phase_limit >= 7:
                            wout_half(0)
                    if phase_limit >= 5:
                        phase4_pre(NSL - 2)
                        ln_finalize(pool_ph4, st2, rstd2, nmr2, 2 * CPC, CPC,
                                    "c2")
                        for i in range(2 * CPC, 3 * CPC):
                            z2_chunk(i)
                        phase4_pre(NSL - 1)
                        ln_finalize(pool_ph4, st2, rstd2, nmr2, 3 * CPC, CPC,
                                    "c3")
                        for i in range(3 * CPC, NT2):
                            z2_chunk(i)
                    if phase_limit >= 6:
                        fc_ts(1)
                        if phase_limit >= 7:
                            wout_half(1)
                    if debug_outs:
                        nc.sync.dma_start(dbg["dbg_x2"][:], X2[:])
                        nc.sync.dma_start(dbg["dbg_z2t"][:], Z2T[:])
                es_z2t.close()

    nc.compile()
    return nc


def _prep_core_inputs(b, parity, x, ln1_w, ln1_b, w_qkv, b_qkv, w_o, b_o,
                      ln2_w, ln2_b, w_fc, b_fc, w_out, b_out,
                      T_, C_, H_, D_):
    """Host-side per-core input dict (weights LN-folded, matmul inputs bf16)."""
    bf16 = ml_dtypes.bfloat16
    HH = H_ // 2
    QH = HH * D_
    T2 = T_ // 2
    wq_eff = (ln1_w[:, None] * w_qkv).astype(np.float32)
    bq_eff = (b_qkv + ln1_b @ w_qkv).astype(np.float32)
    wfc_eff = (ln2_w[:, None] * w_fc).astype(np.float32)
    bfc_eff = (b_fc + ln2_b @ w_fc).astype(np.float32)

    h0 = parity * QH
    sl_q = slice(h0, h0 + QH)
    sl_k = slice(C_ + h0, C_ + h0 + QH)
    sl_v = slice(2 * C_ + h0, 2 * C_ + h0 + QH)
    tri = np.tril(np.ones((P, P), np.float32)).T  # tri[k,q] = 1 if k <= q
    ident = np.eye(P, dtype=np.float32)
    SL_ = min(512, T_)
    HS = SL_ // 2
    own_rows = np.concatenate([
        np.arange(s * SL_ + parity * HS, s * SL_ + (parity + 1) * HS)
        for s in range(T_ // SL_)])
    bo_eff = b_o if parity == 0 else np.zeros_like(b_o)
    return {
        "x_full": np.ascontiguousarray(x[b]).astype(bf16),
        "x_own": np.ascontiguousarray(x[b, own_rows]),
        "wq": np.ascontiguousarray(wq_eff[:, sl_q]).astype(bf16),
        "wk": np.ascontiguousarray(wq_eff[:, sl_k]).astype(bf16),
        "wv": np.ascontiguousarray(wq_eff[:, sl_v]).astype(bf16),
        "bq": np.ascontiguousarray(bq_eff[sl_q]),
        "bv": np.ascontiguousarray(bq_eff[sl_v]),
        "wo": np.ascontiguousarray(w_o[h0 : h0 + QH, :]).astype(bf16),
        "bo": np.ascontiguousarray(bo_eff).astype(bf16),
        "wfc": np.ascontiguousarray(wfc_eff).astype(bf16),
        "bfc": np.ascontiguousarray(bfc_eff),
        "wout": np.ascontiguousarray(w_out).astype(bf16),
        "bout": np.ascontiguousarray(b_out).astype(bf16),
        "tri": tri.astype(bf16),
        "ident": ident.astype(bf16),
    }


def kernel(x, ln1_w, ln1_b, w_qkv, b_qkv, w_o, b_o, ln2_w, ln2_b,
           w_fc, b_fc, w_out, b_out):
    from concourse.bass_utils import run_bass_kernel_spmd

    key = (T, C, H, D, FF, N_CORES)
    if key not in _CACHE:
        groups = [[2 * i, 2 * i + 1] for i in range(N_CORES // 2)]
        _CACHE[key] = _build(T, C, H, D, FF, N_CORES, groups)
    nc = _CACHE[key]

    args = (np.asarray(x, np.float32), np.asarray(ln1_w, np.float32),
            np.asarray(ln1_b, np.float32), np.asarray(w_qkv, np.float32),
            np.asarray(b_qkv, np.float32), np.asarray(w_o, np.float32),
            np.asarray(b_o, np.float32), np.asarray(ln2_w, np.float32),
            np.asarray(ln2_b, np.float32), np.asarray(w_fc, np.float32),
            np.asarray(b_fc, np.float32), np.asarray(w_out, np.float32),
            np.asarray(b_out, np.float32))
    in_maps = []
    for core in range(N_CORES):
        b, parity = core // 2, core % 2
        in_maps.append(_prep_core_inputs(b, parity, *args, T, C, H, D))

    global LAST_RESULT
    res = run_bass_kernel_spmd(nc, in_maps, core_ids=list(range(N_CORES)))
    LAST_RESULT = res

    SL_ = min(512, T)
    HS = SL_ // 2
    full = np.empty((B, T, C), np.float32)
    for core in range(N_CORES):
        b, parity = core // 2, core % 2
        o = res.results[core]["out"]
        for s in range(T // SL_):
            full[b, s * SL_ + parity * HS : s * SL_ + (parity + 1) * HS] = \
                o[s * HS : (s + 1) * HS]
    return full
